# revision 1
# baseline (speedup 1.0000x reference)
"""Conv2d 3x3 VALID stride-1 kernel for Trainium2 (Bass/Tile), 8-core SPMD.

x: [32, 128, 112, 112] f32, weight: [256, 128, 3, 3] f32
out: [32, 256, 110, 110] f32

Strategy: implicit GEMM. Cin=128 sits on the SBUF partition dim and is the
matmul contraction axis. For each of the 9 filter taps (kh, kw), a matmul
with lhsT = weight[ci, co_tile] and rhs = x[ci, shifted-window pixels]
accumulates into PSUM (start on tap 0, stop on tap 8). Output row-chunks
of 4 rows (free dim 440 <= 512 fp32 = one PSUM bank) stream through the
PE at 1 cycle/row. Inputs are cast to fp16 on the way into SBUF (same
10-bit mantissa as TF32 -> rel err ~3e-4 on this data, but LDWEIGHTS is
2x faster than fp32r and hides completely under the matmul stream).
Data-parallel over batch: 4 images per core, weights replicated.

Measured on 8xNC-v3 (axon): ~390 us NEFF exec, ~93% of the 363.6 us
PE-MAC roofline. rel err (Frobenius) 2.9e-4 vs the fp32 jax reference.
"""

import numpy as np

import concourse.mybir as mybir
import concourse.tile as tile
from concourse import bacc
from concourse.bass_utils import run_bass_kernel_spmd

B, CIN, H, W = 32, 128, 112, 112
COUT, KH, KW = 256, 3, 3
OH, OW = H - KH + 1, W - KW + 1  # 110, 110
NCORES = 8
BPC = B // NCORES  # batches per core

F32 = mybir.dt.float32
F32R = mybir.dt.float32r
BF16 = mybir.dt.bfloat16

# Compute dtype for the TensorEngine inputs, all HW-measured on this conv:
#   fp16 (default): 186 ns/MM, rel err 2.9e-4 (10-bit mantissa, range OK
#                   for randn data; LDWEIGHTS 97 ns hides under the stream)
#   f32r:           200 ns/MM, rel err 1.5e-4 (TF32; LDWEIGHTS 187 ns adds
#                   ~14 ns/MM that cannot be hidden)
#   bf16:           186 ns/MM, rel err 2.4e-3
import os as _os
FP16 = mybir.dt.float16
_DT_MAP = {"f32r": F32R, "bf16": BF16, "fp16": FP16}
COMPUTE_DT = _DT_MAP[_os.environ.get("CONV_DT", "fp16")]

# Row-chunking of the 110 output rows: free dim = rows*110, must be <= 512
# (PSUM bank) and >= 256 (fp32r full-rate threshold). 26*4 + 2*3 = 110.
ROW_CHUNKS = [4] * 26 + [3] * 2

_CACHE = {}


def _build_nc():
    nc = bacc.Bacc("TRN2", target_bir_lowering=False, debug=False)

    x_d = nc.dram_tensor("x", [BPC, CIN, H, W], F32, kind="ExternalInput")
    w_d = nc.dram_tensor("w", [CIN, KH * KW, COUT], F32, kind="ExternalInput")
    o_d = nc.dram_tensor("o", [BPC, COUT, OH, OW], F32, kind="ExternalOutput")

    from concourse.bass import _add_dep_helper

    xbufs = 2 if COMPUTE_DT == F32R else 3
    # Prefetch chunking of images b >= 1 (14-row pieces), paced against the
    # previous batch's compute so the SWDGE input stream never bursts hard
    # enough to starve the HWDGE output stores of SDMA bandwidth.
    PF_BOUNDS = [0, 14, 28, 42, 56, 70, 84, 98, 112]
    N_GROUPS = 2 * len(ROW_CHUNKS)  # (row-chunk, ct) groups per batch

    with tile.TileContext(nc) as tc:
        with (
            tc.tile_pool(name="wpool", bufs=1) as wpool,
            tc.tile_pool(name="xpool", bufs=xbufs) as xpool,
            tc.tile_pool(name="opool", bufs=16) as opool,
            tc.tile_pool(name="psum", bufs=8, space="PSUM") as psum,
        ):
            # PE pre-warm: dependency-free dummy matmuls on a never-written
            # scratch tile keep the PE busy from engine boot until the first
            # real matmul's data arrives, so the HAM clock gate is already
            # at 2.4 GHz (warm) when real work starts and the ~3 us
            # half-clock ramp is paid on garbage instead.
            scratch = wpool.tile([128, 512], COMPUTE_DT, name="warm_scratch")
            nc.vector.memset(scratch[:], 0)
            ps_warm = psum.tile([128, 512], F32, name="warm_psum", tag="ps")
            for _ in range(16):
                nc.tensor.matmul(
                    ps_warm[:], scratch[:, 0:128], scratch[:],
                    start=True, stop=True, skip_group_check=True,
                )

            wr = wpool.tile([CIN, KH * KW, COUT], COMPUTE_DT)
            # ct=0's weight columns first: the first matmuls need only them.
            nc.gpsimd.dma_start(wr[:, :, 0:128], w_d[:, :, 0:128])

            # Image 0: load immediately (it gates the first matmuls). Small
            # leading chunk = exactly the rows the first matmul group reads.
            xtiles = [xpool.tile([CIN, H, W], COMPUTE_DT, tag="x", name="x0")]
            for r0, r1 in zip(b0 := [0, 6, 16, 28, 42, 56, 70, 84, 98, 112], b0[1:]):
                nc.gpsimd.dma_start(
                    xtiles[0][:, r0:r1, :], x_d[0, :, r0:r1, :]
                )
                if r1 == 6:
                    nc.gpsimd.dma_start(wr[:, :, 128:256], w_d[:, :, 128:256])

            for b in range(BPC):
                xr = xtiles[b]
                if b + 1 < BPC:
                    xtiles.append(
                        xpool.tile(
                            [CIN, H, W], COMPUTE_DT, tag="x", name=f"x{b+1}"
                        )
                    )
                # Milestone group index at which to release prefetch chunk j
                # of image b+1: spread the 8 chunks across this batch.
                pf_at = {
                    (N_GROUPS * j) // len(PF_BOUNDS[1:]): j
                    for j in range(len(PF_BOUNDS) - 1)
                }

                # Interleave the two cout-tiles per row-chunk: halves the
                # x-row consumption rate so compute never overruns the
                # image DMA at kernel start.
                oh = 0
                gidx = 0
                for R in ROW_CHUNKS:
                    for ct in range(2):
                        co0 = ct * 128
                        ps = psum.tile([128, R, OW], F32, tag="ps")
                        for idx in range(KH * KW):
                            kh, kw = divmod(idx, KW)
                            nc.tensor.matmul(
                                ps[:],
                                wr[:, idx, co0 : co0 + 128],
                                xr[:, oh + kh : oh + kh + R, kw : kw + OW],
                                start=(idx == 0),
                                stop=(idx == KH * KW - 1),
                            )
                        ot = opool.tile([128, R, OW], F32, tag="ot")
                        cp = nc.vector.tensor_copy(ot[:], ps[:])
                        nc.sync.dma_start(
                            o_d[b, co0 : co0 + 128, oh : oh + R, :], ot[:]
                        )
                        if b + 1 < BPC and gidx in pf_at:
                            j = pf_at[gidx]
                            r0, r1 = PF_BOUNDS[j], PF_BOUNDS[j + 1]
                            dma = nc.gpsimd.dma_start(
                                xtiles[b + 1][:, r0:r1, :],
                                x_d[b + 1, :, r0:r1, :],
                            )
                            _add_dep_helper(
                                dma.ins,
                                cp.ins,
                                sync=True,
                                reason="pace input prefetch vs compute",
                            )
                        gidx += 1
                    oh += R

    nc.compile()
    return nc


def _get_nc():
    if "nc" not in _CACHE:
        _CACHE["nc"] = _build_nc()
    return _CACHE["nc"]


LAST_RESULT = None


def kernel(x, weight, trace=False):
    global LAST_RESULT
    x = np.ascontiguousarray(np.asarray(x, dtype=np.float32))
    weight = np.asarray(weight, dtype=np.float32)
    # [Cout, Cin, kh, kw] -> [Cin, kh*kw, Cout], contiguous
    w_packed = np.ascontiguousarray(
        weight.transpose(1, 2, 3, 0).reshape(CIN, KH * KW, COUT)
    )

    nc = _get_nc()
    in_maps = [
        {"x": x[i * BPC : (i + 1) * BPC], "w": w_packed} for i in range(NCORES)
    ]
    res = run_bass_kernel_spmd(
        nc, in_maps, core_ids=list(range(NCORES)), trace=trace
    )
    LAST_RESULT = res
    out = np.concatenate([r["o"] for r in res.results], axis=0)
    return out



# revision 2
# speedup vs baseline: 1.0151x; 1.0151x over previous
"""Conv2d 3x3 VALID stride-1 for Trainium2 (Bass/Tile), 8-core SPMD.
1-D Winograd F(2,3) along H: 12 matmuls per 8 output rows instead of 18.

x: [32, 128, 112, 112] f32, weight: [256, 128, 3, 3] f32
out: [32, 256, 110, 110] f32

Per output-row-pair t (55 pairs/image), with d = x rows [2t..2t+3]:
  t0 = d0-d2, t1 = d1+d2, t2 = d2-d1, t3 = d1-d3          (DVE, fp16, 2x)
  m_i = sum_kw Gw[:,kw,i,:] @ t_i[:, t, kw:kw+110]         (PE, 3 taps into PSUM bank i)
  o[2t]   = m0+m1+m2       o[2t+1] = m1-m2-m3              (ScalarE evacuates m0..m2 to
                                                            fp16, DVE combines; o fp16)
Weight transform Gw = G @ w over kh is precomputed on host; the fp16
output is upcast to f32 on host. Data-parallel over batch: 4 img/core.
H rows are viewed as [56, 2] (pair, parity) so the stride-2 row reads of
the transform are plain slices.
"""

from collections import deque

import numpy as np

import concourse.mybir as mybir
import concourse.tile as tile
from concourse import bacc
from concourse.bass_utils import run_bass_kernel_spmd

B, CIN, H, W = 32, 128, 112, 112
COUT, KH, KW = 256, 3, 3
OH, OW = H - KH + 1, W - KW + 1  # 110, 110
NCORES = 8
BPC = B // NCORES  # images per core
HP = H // 2  # 56 row-pairs of input

NT = OH // 2            # 55 output row-pairs per image
TB = 4                  # row-pairs per block (PSUM bank: 4*110=440 <= 512)
BLOCKS = [(i * TB, TB) for i in range(NT // TB)] + [(NT - NT % TB, NT % TB)]
# -> 13 blocks of 4 + 1 block of 3
N_BCT = len(BLOCKS) * 2  # block-cts per image

# input-transform chunking: 8 chunks; a small first chunk (exactly the
# pairs block 0 needs) shortens the startup critical path. Chunk c covers
# output pairs [TCH[c]) and needs input row-pairs up to TCH[c][1]+1.
TCH = [(0, 4)] + [(8 * c - 4, min(8 * c + 4, NT)) for c in range(1, 8)]
# disjoint x DMA chunks (input row-pairs); chunk c covers what transform
# chunk c needs beyond chunk c-1
XCH = [(0, 5)] + [(8 * c - 3, min(8 * c + 5, HP)) for c in range(1, 8)]

# transform op i -> (pair_shift_a, parity_a, pair_shift_b, parity_b, op):
#   t_i[pair t] = x[2t + a] op x[2t + b],  row 2t+d -> (pair t + d//2, d%2)
_TOPS = [
    (0, 0, 1, 0, "subtract"),  # t0 = d0 - d2
    (0, 1, 1, 0, "add"),       # t1 = d1 + d2
    (1, 0, 0, 1, "subtract"),  # t2 = d2 - d1
    (0, 1, 1, 1, "subtract"),  # t3 = d1 - d3
]

F32 = mybir.dt.float32
FP16 = mybir.dt.float16

_CACHE = {}


def _build_nc():
    nc = bacc.Bacc("TRN2", target_bir_lowering=False, debug=False)
    OP = mybir.AluOpType

    x_d = nc.dram_tensor("x", [BPC, CIN, HP, 2, W], FP16, kind="ExternalInput")
    w_d = nc.dram_tensor("w", [CIN, KW, 4, COUT], FP16, kind="ExternalInput")
    # [.., 55, 2, 110]: output row 2t+p lives at [t, p] -> even/odd stores
    # are plain slices
    o_d = nc.dram_tensor("o", [BPC, COUT, NT, 2, OW], FP16, kind="ExternalOutput")

    from concourse.bass import _add_dep_helper

    with tile.TileContext(nc) as tc:
        with (
            tc.tile_pool(name="wpool", bufs=1) as wpool,
            tc.tile_pool(name="xpool", bufs=2) as xpool,
            tc.tile_pool(name="tpool", bufs=2) as tpool,
            tc.tile_pool(name="cpool", bufs=2) as cpool,
            tc.tile_pool(name="opool", bufs=8) as opool,
            tc.tile_pool(name="psum", bufs=8, space="PSUM") as psum,
        ):
            # PE pre-warm on garbage so the HAM clock gate is at 2.4 GHz
            # by the time real matmuls start.
            wscr = wpool.tile([128, 128], FP16, name="warm_w")
            xscr = wpool.tile([128, 4, 110], FP16, name="warm_x")
            nc.vector.memset(wscr[:], 0)
            nc.vector.memset(xscr[:], 0)
            ps_warm = psum.tile([128, 4, 110], F32, name="warm_psum", tag="ps")
            for _ in range(16):
                nc.tensor.matmul(
                    ps_warm[:, 0:4, :], wscr[:], xscr[:],
                    start=True, stop=True, skip_group_check=True,
                )

            wr = wpool.tile([CIN, KW, 4, COUT], FP16)
            xts = [xpool.tile([CIN, HP, 2, W], FP16, tag="x", name="x0")]
            tts = [tpool.tile([CIN, 4, NT, W], FP16, tag="t", name="t0")]
            # x chunk 0 first: it gates the first transform ops + matmuls
            p0, p1 = XCH[0]
            nc.gpsimd.dma_start(xts[0][:, p0:p1, :, :], x_d[0, :, p0:p1, :, :])
            nc.gpsimd.dma_start(wr[:, :, :, 0:128], w_d[:, :, :, 0:128])
            nc.gpsimd.dma_start(wr[:, :, :, 128:256], w_d[:, :, :, 128:256])
            for p0, p1 in XCH[1:]:
                nc.gpsimd.dma_start(
                    xts[0][:, p0:p1, :, :], x_d[0, :, p0:p1, :, :]
                )

            def temit(b, j):
                """Input-transform op j (chunk j//4, i=j%4) for image b."""
                c, i = divmod(j, 4)
                t0, t1 = TCH[c]
                xr, tr = xts[b], tts[b]
                sa, pa, sb, pb, opname = _TOPS[i]
                nc.vector.tensor_tensor(
                    tr[:, i, t0:t1, :],
                    xr[:, t0 + sa : t1 + sa, pa, :],
                    xr[:, t0 + sb : t1 + sb, pb, :],
                    getattr(OP, opname),
                )

            # pending transform ops: (image, op j, min global bct to issue)
            NOPS = 4 * len(TCH)
            pending = deque()
            for j in range(12):
                temit(0, j)  # front-loaded; blocks 0..4 covered
            for j in range(12, NOPS):
                pending.append((0, j, (j - 12) // 2))
            for b in range(1, BPC):
                for j in range(NOPS):
                    # x chunk j//4 of image b released at global bct
                    # (b-1)*N_BCT + 3*(j//4); +2 bcts for the DMA to land
                    pending.append((b, j, (b - 1) * N_BCT + 3 * (j // 4) + 2))

            gbct = 0
            for b in range(BPC):
                tr = tts[b]
                if b + 1 < BPC:
                    xts.append(
                        xpool.tile([CIN, HP, 2, W], FP16, tag="x", name=f"x{b+1}")
                    )
                    tts.append(
                        tpool.tile([CIN, 4, NT, W], FP16, tag="t", name=f"t{b+1}")
                    )
                for tb0, T in BLOCKS:
                    for ct in range(2):
                        co0 = ct * 128
                        ms = [
                            psum.tile([128, 4, 110], F32, tag="ps", name=f"m{i}")
                            for i in range(4)
                        ]
                        cs = []
                        for i in range(4):
                            for kw in range(KW):
                                nc.tensor.matmul(
                                    ms[i][:, 0:T, :],
                                    wr[:, kw, i, co0 : co0 + 128],
                                    tr[:, i, tb0 : tb0 + T, kw : kw + OW],
                                    start=(kw == 0),
                                    stop=(kw == KW - 1),
                                )
                            if i < 3:
                                ci = cpool.tile(
                                    [128, TB, OW], FP16, tag=f"c{i}",
                                    name=f"c{i}",
                                )
                                nc.scalar.copy(ci[:, 0:T, :], ms[i][:, 0:T, :])
                                cs.append(ci)
                        c0, c1, c2 = cs
                        ts_ = cpool.tile([128, TB, OW], FP16, tag="ts", name="ts_")
                        us_ = cpool.tile([128, TB, OW], FP16, tag="us", name="us_")
                        nc.vector.tensor_tensor(
                            ts_[:, 0:T, :], c1[:, 0:T, :], c2[:, 0:T, :], OP.add)
                        nc.vector.tensor_tensor(
                            us_[:, 0:T, :], c1[:, 0:T, :], c2[:, 0:T, :],
                            OP.subtract)
                        ot = opool.tile([128, TB, 2, OW], FP16, tag="ot", name="ot")
                        cpe = nc.vector.tensor_tensor(
                            ot[:, 0:T, 0, :], ts_[:, 0:T, :], c0[:, 0:T, :],
                            OP.add)
                        nc.vector.tensor_tensor(
                            ot[:, 0:T, 1, :], us_[:, 0:T, :], ms[3][:, 0:T, :],
                            OP.subtract)
                        nc.sync.dma_start(
                            o_d[b, co0 : co0 + 128, tb0 : tb0 + T, :, :],
                            ot[:, 0:T, :, :])

                        # paced successor-image x streaming
                        bct = gbct - b * N_BCT
                        if b + 1 < BPC and bct % 3 == 0 and bct // 3 < len(XCH):
                            p0, p1 = XCH[bct // 3]
                            dma = nc.gpsimd.dma_start(
                                xts[b + 1][:, p0:p1, :, :],
                                x_d[b + 1, :, p0:p1, :, :],
                            )
                            _add_dep_helper(
                                dma.ins, cpe.ins, sync=True,
                                reason="pace x prefetch vs compute",
                            )
                        # drain up to 2 eligible transform ops
                        popped = 0
                        while pending and popped < 2 and pending[0][2] <= gbct:
                            tb_, tj, _ = pending.popleft()
                            temit(tb_, tj)
                            popped += 1
                        gbct += 1
            # any leftovers (shouldn't happen)
            while pending:
                tb_, tj, _ = pending.popleft()
                temit(tb_, tj)

    nc.compile()
    return nc


def _get_nc():
    if "nc" not in _CACHE:
        _CACHE["nc"] = _build_nc()
    return _CACHE["nc"]


LAST_RESULT = None

_G = np.array(
    [[1, 0, 0], [0.5, 0.5, 0.5], [0.5, -0.5, 0.5], [0, 0, 1]], np.float32
)


def kernel(x, weight, trace=False):
    global LAST_RESULT
    x16 = np.asarray(x, dtype=np.float32).astype(np.float16)
    x16 = x16.reshape(B, CIN, HP, 2, W)
    w32 = np.asarray(weight, dtype=np.float32)
    # Gw[cin, kw, i, cout] = sum_kh G[i, kh] * w[cout, cin, kh, kw]
    gw = np.einsum("ik,ockw->cwio", _G, w32).astype(np.float16)
    gw = np.ascontiguousarray(gw)

    nc = _get_nc()
    in_maps = [
        {"x": x16[i * BPC : (i + 1) * BPC], "w": gw} for i in range(NCORES)
    ]
    res = run_bass_kernel_spmd(
        nc, in_maps, core_ids=list(range(NCORES)), trace=trace
    )
    LAST_RESULT = res
    out = np.concatenate(
        [r["o"].reshape(BPC, COUT, OH, OW) for r in res.results], axis=0
    )
    return out.astype(np.float32)


# revision 3
# speedup vs baseline: 1.1640x; 1.1467x over previous
"""Conv2d 3x3 VALID stride-1 for Trainium2 (Bass/Tile), 8-core SPMD.
1-D Winograd F(2,3) along H: 12 matmuls per 8 output rows instead of 18.

x: [32, 128, 112, 112] f32, weight: [256, 128, 3, 3] f32
out: [32, 256, 110, 110] f32

Per output-row-pair t (55 pairs/image), with d = x rows [2t..2t+3]:
  t0 = d0-d2, t1 = d1+d2, t2 = d2-d1, t3 = d1-d3          (DVE, fp16, 2x)
  m_i = sum_kw Gw[:,kw,i,:] @ t_i[:, t, kw:kw+110]         (PE, 3 taps into PSUM bank i)
  o[2t]   = m0+m1+m2       o[2t+1] = m1-m2-m3              (DVE, all fp16)
m0..m3 live in two 2-bank PSUM tiles; ScalarE evacuates each pair with
one strided copy to fp16 SBUF as soon as its second bank stops, so the
PE never waits on PSUM recycling and the DVE never reads PSUM. Weight
transform Gw = G @ w over kh is precomputed on host; the fp16 output is
upcast to f32 on host. Data-parallel over batch: 4 img/core. H rows are
viewed as [56, 2] (pair, parity) so the stride-2 row reads of the
transform are plain slices.

Measured on 8xNC-v3 (axon): ~266 us NEFF exec on a cool chip (~320 us
when the chip is P0 power-throttled to 5/6 clock); matmul stream runs at
~185.5 ns per 440-free-dim fp16 matmul (~99% of the 183.3 ns roofline).
rel err (Frobenius) 6.1e-4 vs the fp32 jax reference.
"""

from collections import deque

import numpy as np

import concourse.mybir as mybir
import concourse.tile as tile
from concourse import bacc
from concourse.bass_utils import run_bass_kernel_spmd

B, CIN, H, W = 32, 128, 112, 112
COUT, KH, KW = 256, 3, 3
OH, OW = H - KH + 1, W - KW + 1  # 110, 110
NCORES = 8
BPC = B // NCORES  # images per core
HP = H // 2  # 56 row-pairs of input

NT = OH // 2            # 55 output row-pairs per image
TB = 4                  # row-pairs per block (PSUM bank: 4*110=440 <= 512)
BLOCKS = [(i * TB, TB) for i in range(NT // TB)] + [(NT - NT % TB, NT % TB)]
# -> 13 blocks of 4 + 1 block of 3
N_BCT = len(BLOCKS) * 2  # block-cts per image

# input-transform chunking: 8 chunks; a small first chunk (exactly the
# pairs block 0 needs) shortens the startup critical path. Chunk c covers
# output pairs [TCH[c]) and needs input row-pairs up to TCH[c][1]+1.
TCH = [(0, 4)] + [(8 * c - 4, min(8 * c + 4, NT)) for c in range(1, 8)]
# disjoint x DMA chunks (input row-pairs); chunk c covers what transform
# chunk c needs beyond chunk c-1
XCH = [(0, 5)] + [(8 * c - 3, min(8 * c + 5, HP)) for c in range(1, 8)]

# transform op i -> (pair_shift_a, parity_a, pair_shift_b, parity_b, op):
#   t_i[pair t] = x[2t + a] op x[2t + b],  row 2t+d -> (pair t + d//2, d%2)
_TOPS = [
    (0, 0, 1, 0, "subtract"),  # t0 = d0 - d2
    (0, 1, 1, 0, "add"),       # t1 = d1 + d2
    (1, 0, 0, 1, "subtract"),  # t2 = d2 - d1
    (0, 1, 1, 1, "subtract"),  # t3 = d1 - d3
]

F32 = mybir.dt.float32
FP16 = mybir.dt.float16

_CACHE = {}


def _build_nc():
    nc = bacc.Bacc("TRN2", target_bir_lowering=False, debug=False)
    OP = mybir.AluOpType

    x_d = nc.dram_tensor("x", [BPC, CIN, HP, 2, W], FP16, kind="ExternalInput")
    w_d = nc.dram_tensor("w", [CIN, KW, 4, COUT], FP16, kind="ExternalInput")
    # [.., 55, 2, 110]: output row 2t+p lives at [t, p] -> even/odd stores
    # are plain slices
    o_d = nc.dram_tensor("o", [BPC, COUT, NT, 2, OW], FP16, kind="ExternalOutput")

    from concourse.bass import _add_dep_helper

    with tile.TileContext(nc) as tc:
        with (
            tc.tile_pool(name="wpool", bufs=1) as wpool,
            tc.tile_pool(name="xpool", bufs=2) as xpool,
            tc.tile_pool(name="tpool", bufs=2) as tpool,
            tc.tile_pool(name="cpool", bufs=2) as cpool,
            tc.tile_pool(name="opool", bufs=8) as opool,
            tc.tile_pool(name="psum", bufs=8, space="PSUM") as psum,
        ):
            # PE pre-warm on garbage so the HAM clock gate is at 2.4 GHz
            # by the time real matmuls start.
            wscr = wpool.tile([128, 128], FP16, name="warm_w")
            xscr = wpool.tile([128, 4, 110], FP16, name="warm_x")
            nc.vector.memset(wscr[:], 0)
            nc.vector.memset(xscr[:], 0)
            ps_warm = psum.tile([128, 2, 4, 128], F32, name="warm_psum", tag="psA", bufs=2)
            for _ in range(16):
                nc.tensor.matmul(
                    ps_warm[:, 0, 0:4, 0:110], wscr[:], xscr[:],
                    start=True, stop=True, skip_group_check=True,
                )

            wr = wpool.tile([CIN, KW, 4, COUT], FP16)
            xts = [xpool.tile([CIN, HP, 2, W], FP16, tag="x", name="x0")]
            tts = [tpool.tile([CIN, 4, NT, W], FP16, tag="t", name="t0")]
            # x chunk 0 first: it gates the first transform ops + matmuls
            p0, p1 = XCH[0]
            nc.gpsimd.dma_start(xts[0][:, p0:p1, :, :], x_d[0, :, p0:p1, :, :])
            nc.gpsimd.dma_start(wr[:, :, :, 0:128], w_d[:, :, :, 0:128])
            nc.gpsimd.dma_start(wr[:, :, :, 128:256], w_d[:, :, :, 128:256])
            for p0, p1 in XCH[1:]:
                nc.gpsimd.dma_start(
                    xts[0][:, p0:p1, :, :], x_d[0, :, p0:p1, :, :]
                )

            def temit(b, j):
                """Input-transform op j (chunk j//4, i=j%4) for image b."""
                c, i = divmod(j, 4)
                t0, t1 = TCH[c]
                xr, tr = xts[b], tts[b]
                sa, pa, sb, pb, opname = _TOPS[i]
                nc.vector.tensor_tensor(
                    tr[:, i, t0:t1, :],
                    xr[:, t0 + sa : t1 + sa, pa, :],
                    xr[:, t0 + sb : t1 + sb, pb, :],
                    getattr(OP, opname),
                )

            # pending transform ops: (image, op j, min global bct to issue)
            NOPS = 4 * len(TCH)
            pending = deque()
            for j in range(12):
                temit(0, j)  # front-loaded; blocks 0..4 covered
            for j in range(12, NOPS):
                pending.append((0, j, (j - 12) // 2))
            for b in range(1, BPC):
                for j in range(NOPS):
                    # x chunk j//4 of image b released at global bct
                    # (b-1)*N_BCT + 3*(j//4); +2 bcts for the DMA to land
                    pending.append((b, j, (b - 1) * N_BCT + 3 * (j // 4) + 2))

            gbct = 0
            for b in range(BPC):
                tr = tts[b]
                if b + 1 < BPC:
                    xts.append(
                        xpool.tile([CIN, HP, 2, W], FP16, tag="x", name=f"x{b+1}")
                    )
                    tts.append(
                        tpool.tile([CIN, 4, NT, W], FP16, tag="t", name=f"t{b+1}")
                    )
                for tb0, T in BLOCKS:
                    for ct in range(2):
                        co0 = ct * 128
                        msA = psum.tile([128, 2, 4, 128], F32, tag="psA", name="msA", bufs=2)
                        msB = psum.tile([128, 2, 4, 128], F32, tag="psB", name="msB", bufs=2)
                        cas = []
                        for i in range(4):
                            mtile = msA if i < 2 else msB
                            for kw in range(KW):
                                nc.tensor.matmul(
                                    mtile[:, i % 2, 0:T, 0:110],
                                    wr[:, kw, i, co0 : co0 + 128],
                                    tr[:, i, tb0 : tb0 + T, kw : kw + OW],
                                    start=(kw == 0),
                                    stop=(kw == KW - 1),
                                )
                            if i % 2 == 1:
                                ci = cpool.tile(
                                    [128, 2, TB, OW], FP16, tag=f"ca{i//2}",
                                    name=f"ca{i//2}",
                                )
                                nc.scalar.copy(
                                    ci[:, :, 0:T, :], mtile[:, :, 0:T, 0:110]
                                )
                                cas.append(ci)
                        c0 = cas[0][:, 0, 0:T, :]
                        c1 = cas[0][:, 1, 0:T, :]
                        c2 = cas[1][:, 0, 0:T, :]
                        c3 = cas[1][:, 1, 0:T, :]
                        ts_ = cpool.tile([128, TB, OW], FP16, tag="ts", name="ts_")
                        us_ = cpool.tile([128, TB, OW], FP16, tag="us", name="us_")
                        nc.vector.tensor_tensor(ts_[:, 0:T, :], c1, c2, OP.add)
                        nc.vector.tensor_tensor(us_[:, 0:T, :], c1, c2, OP.subtract)
                        ot = opool.tile([128, TB, 2, OW], FP16, tag="ot", name="ot")
                        cpe = nc.vector.tensor_tensor(
                            ot[:, 0:T, 0, :], ts_[:, 0:T, :], c0, OP.add)
                        nc.vector.tensor_tensor(
                            ot[:, 0:T, 1, :], us_[:, 0:T, :], c3, OP.subtract)
                        nc.sync.dma_start(
                            o_d[b, co0 : co0 + 128, tb0 : tb0 + T, :, :],
                            ot[:, 0:T, :, :])

                        # paced successor-image x streaming
                        bct = gbct - b * N_BCT
                        if b + 1 < BPC and bct % 3 == 0 and bct // 3 < len(XCH):
                            p0, p1 = XCH[bct // 3]
                            dma = nc.gpsimd.dma_start(
                                xts[b + 1][:, p0:p1, :, :],
                                x_d[b + 1, :, p0:p1, :, :],
                            )
                            _add_dep_helper(
                                dma.ins, cpe.ins, sync=True,
                                reason="pace x prefetch vs compute",
                            )
                        # drain up to 2 eligible transform ops
                        popped = 0
                        while pending and popped < 2 and pending[0][2] <= gbct:
                            tb_, tj, _ = pending.popleft()
                            temit(tb_, tj)
                            popped += 1
                        gbct += 1
            # any leftovers (shouldn't happen)
            while pending:
                tb_, tj, _ = pending.popleft()
                temit(tb_, tj)

    nc.compile()
    return nc


def _get_nc():
    if "nc" not in _CACHE:
        _CACHE["nc"] = _build_nc()
    return _CACHE["nc"]


LAST_RESULT = None

_G = np.array(
    [[1, 0, 0], [0.5, 0.5, 0.5], [0.5, -0.5, 0.5], [0, 0, 1]], np.float32
)


def kernel(x, weight, trace=False):
    global LAST_RESULT
    x16 = np.asarray(x, dtype=np.float32).astype(np.float16)
    x16 = x16.reshape(B, CIN, HP, 2, W)
    w32 = np.asarray(weight, dtype=np.float32)
    # Gw[cin, kw, i, cout] = sum_kh G[i, kh] * w[cout, cin, kh, kw]
    gw = np.einsum("ik,ockw->cwio", _G, w32).astype(np.float16)
    gw = np.ascontiguousarray(gw)

    nc = _get_nc()
    in_maps = [
        {"x": x16[i * BPC : (i + 1) * BPC], "w": gw} for i in range(NCORES)
    ]
    res = run_bass_kernel_spmd(
        nc, in_maps, core_ids=list(range(NCORES)), trace=trace
    )
    LAST_RESULT = res
    out = np.concatenate(
        [r["o"].reshape(BPC, COUT, OH, OW) for r in res.results], axis=0
    )
    return out.astype(np.float32)
